# revision 12
# baseline (speedup 1.0000x reference)
"""Multi-head cross-attention (LN + QKV proj + masked/scaled softmax attention
+ output proj) on 8 Trainium2 NeuronCores.

Sharding: core c handles (batch b = c//2, head-group hg = c%2).  Each core
computes LayerNorm + projections for its batch restricted to its 8 heads
(tensor-parallel split of Wq/Wk/Wv output dim and Wo input dim), full
attention for those heads (att written directly), and a partial output
projection.  Host sums the two partial outputs per batch and adds bo.

Device math (fp32 data, float32r matmuls ~ tf32-class, fp32 accumulate):
  LN affine (g, b) is folded into the projection weights on host:
    LN(x; g, b) @ W + c  ==  LNplain(x) @ (g[:, None] * W) + (b @ W + c)
  aux_score/8 is folded into the per-token LN scale of the k path.
  The key-mask becomes an additive -30000 row via a K=1 matmul so masked
  exp() underflows to exactly 0 (matching reference where(m, 0, att)).
"""
import sys

sys.path.insert(0, "/opt/trn_rl_repo")

import numpy as np

import concourse.bass as bass
import concourse.tile as tile
from concourse import bacc, mybir
F32 = mybir.dt.float32
F32R = mybir.dt.float32r
ActF = mybir.ActivationFunctionType
Alu = mybir.AluOpType

TQ = 512          # q tokens per batch
TK = 2048         # kv tokens per batch
D = 1024          # model dim
HD = 64           # head dim
NH = 8            # heads per core
OD = 512          # projected width per core (8 heads * 64)
EPS = 1e-5

_CACHED_NC = None


def _layer_norm_tiles(nc, pools, x_dram, tok0, ntiles, aux_sb, aux_col0):
    """LN over `ntiles` [128, D] token tiles starting at token tok0.

    Returns list of f32r normalized tiles.  If aux_sb is not None, the
    per-token scale r is multiplied by aux_sb[:, aux_col0 + i] (aux/8 fold).
    """
    xpool, npool, spool, eps_sb = pools
    out = []
    for i in range(ntiles):
        xt = xpool.tile([128, D], F32, tag="ln_x")
        nc.sync.dma_start(xt, x_dram[tok0 + i * 128: tok0 + (i + 1) * 128, :])
        stats = spool.tile([128, 2, 6], F32, tag="ln_stats")
        nc.vector.bn_stats(stats[:, 0], xt[:, 0:512])
        nc.vector.bn_stats(stats[:, 1], xt[:, 512:1024])
        mv = spool.tile([128, 2], F32, tag="ln_mv")
        nc.vector.bn_aggr(mv, stats)
        # r = 1/sqrt(var + eps)  (* aux/8 for the k path)
        r = spool.tile([128, 1], F32, tag="ln_r")
        nc.scalar.activation(r, mv[:, 1:2], ActF.Sqrt, bias=eps_sb)
        nc.vector.reciprocal(r, r)
        if aux_sb is not None:
            nc.vector.tensor_tensor(r, r, aux_sb[:, aux_col0 + i: aux_col0 + i + 1],
                                    Alu.mult)
        # bias = -mean * r
        bls = spool.tile([128, 1], F32, tag="ln_b")
        nc.vector.tensor_scalar(bls, mv[:, 0:1], r, -1.0, Alu.mult, Alu.mult)
        xn = npool.tile([128, D], F32R, tag="ln_xn")
        nc.scalar.activation(xn, xt, ActF.Identity, bias=bls, scale=r)
        out.append(xn)
    return out


def _transpose_block(nc, tpsum, xn_tiles, dstT):
    """Transpose 4 [128, D] f32r tiles into dstT [128, D//128, 512]."""
    ident = nc._ident_f32r
    for ko in range(D // 128):
        pt = tpsum.tile([128, 4, 128], F32R, tag="tps")
        for j, xn in enumerate(xn_tiles):
            nc.tensor.transpose(pt[:, j], xn[:, ko * 128:(ko + 1) * 128], ident)
        nc.any.tensor_copy(dstT[:, ko, :], pt)


def build_nc():
    nc = bacc.Bacc("TRN2", target_bir_lowering=False, num_devices=8)

    xq = nc.dram_tensor("xq", [TQ, D], F32, kind="ExternalInput")
    xk = nc.dram_tensor("xk", [TK, D], F32, kind="ExternalInput")
    xv = nc.dram_tensor("xv", [TK, D], F32, kind="ExternalInput")
    aux8 = nc.dram_tensor("aux8", [TK], F32, kind="ExternalInput")
    maskb = nc.dram_tensor("maskb", [1, TK], F32R, kind="ExternalInput")
    ones1 = nc.dram_tensor("ones1", [1, 128], F32R, kind="ExternalInput")
    identin = nc.dram_tensor("identin", [128, 128], F32R, kind="ExternalInput")
    wq = nc.dram_tensor("wq", [D, OD], F32R, kind="ExternalInput")
    wk = nc.dram_tensor("wk", [D, OD], F32R, kind="ExternalInput")
    wv = nc.dram_tensor("wv", [D, OD], F32R, kind="ExternalInput")
    wo = nc.dram_tensor("wo", [OD, D], F32R, kind="ExternalInput")
    bq = nc.dram_tensor("bq", [OD], F32, kind="ExternalInput")
    bk = nc.dram_tensor("bk", [OD], F32, kind="ExternalInput")
    bv = nc.dram_tensor("bv", [OD], F32, kind="ExternalInput")

    att = nc.dram_tensor("att", [NH, TQ, TK], F32R, kind="ExternalOutput")
    outT = nc.dram_tensor("outT", [D, TQ], F32R, kind="ExternalOutput")

    with tile.TileContext(nc) as tc:
        with (
            tc.tile_pool(name="consts", bufs=1) as consts,
            tc.tile_pool(name="persist", bufs=1) as persist,
        ):
            # ---- constants ----
            eps_sb = consts.tile([128, 1], F32)
            nc.vector.memset(eps_sb, EPS)
            ident = consts.tile([128, 128], F32R)
            nc.sync.dma_start(ident, identin[:, :])
            nc._ident_f32r = ident
            ones_sb = consts.tile([1, 128], F32R)
            nc.sync.dma_start(ones_sb, ones1[:, :])
            aux_sb = consts.tile([128, TK // 128], F32)
            nc.sync.dma_start(aux_sb, aux8.rearrange("(t p) -> p t", p=128))
            bq_sb = consts.tile([128, 4], F32)
            nc.sync.dma_start(bq_sb, bq.rearrange("(o p) -> p o", p=128))
            bk_sb = consts.tile([128, 4], F32)
            nc.sync.dma_start(bk_sb, bk.rearrange("(o p) -> p o", p=128))
            bv_sb = consts.tile([64, NH], F32)
            nc.sync.dma_start(bv_sb, bv.rearrange("(h d) -> d h", d=64))

            # ---- persistent activations ----
            qhT = persist.tile([128, 4, TQ], F32R)      # [od%128, od//128, q]
            khT = persist.tile([128, 4, TK], F32R)      # [od%128, od//128, tk]
            vh = persist.tile([128, TK // 128, OD], F32R)  # [tk%128, tk//128, od]
            ctxT = persist.tile([64, NH, TQ], F32R)     # [hd, head, q]

            # ================= projections =================
            with (
                tc.tile_pool(name="projw", bufs=2) as projw,
                tc.tile_pool(name="lnx", bufs=3) as lnx,
                tc.tile_pool(name="lnn", bufs=4) as lnn,
                tc.tile_pool(name="lns", bufs=8) as lns,
                tc.tile_pool(name="xt_chunk", bufs=2) as xtc,
                tc.tile_pool(name="tpsum", bufs=2, space="PSUM") as tpsum,
                tc.tile_pool(name="ppsum", bufs=2, space="PSUM") as ppsum,
            ):
                # q/k/v weights rotate through 2 shared slots (tag "w")
                wq_sb = projw.tile([128, 8, OD], F32R, tag="w")
                nc.sync.dma_start(wq_sb, wq.rearrange("(ko p) o -> p ko o", p=128))

                ln_pools = (lnx, lnn, lns, eps_sb)

                # ---- q path: LN -> transpose -> qhT = Wq^T @ qn^T (+bq) ----
                qn = _layer_norm_tiles(nc, ln_pools, xq, 0, 4, None, 0)
                qnT = xtc.tile([128, 8, TQ], F32R, tag="xT")
                _transpose_block(nc, tpsum, qn, qnT)
                for od in range(4):
                    ps = ppsum.tile([128, 512], F32, tag="proj")
                    for ko in range(8):
                        nc.tensor.matmul(ps, wq_sb[:, ko, od * 128:(od + 1) * 128],
                                         qnT[:, ko, :],
                                         start=(ko == 0), stop=(ko == 7))
                    nc.any.tensor_scalar_add(qhT[:, od, :], ps,
                                             bq_sb[:, od:od + 1])

                # ---- k path (aux/8 folded), per 512-token chunk ----
                wk_sb = projw.tile([128, 8, OD], F32R, tag="w")
                nc.sync.dma_start(wk_sb, wk.rearrange("(ko p) o -> p ko o", p=128))
                for tc4 in range(4):
                    kn = _layer_norm_tiles(nc, ln_pools, xk, tc4 * 512, 4,
                                           aux_sb, tc4 * 4)
                    knT = xtc.tile([128, 8, 512], F32R, tag="xT")
                    _transpose_block(nc, tpsum, kn, knT)
                    for od in range(4):
                        ps = ppsum.tile([128, 512], F32, tag="proj")
                        for ko in range(8):
                            nc.tensor.matmul(
                                ps, wk_sb[:, ko, od * 128:(od + 1) * 128],
                                knT[:, ko, :], start=(ko == 0), stop=(ko == 7))
                        nc.any.tensor_scalar_add(
                            khT[:, od, tc4 * 512:(tc4 + 1) * 512], ps,
                            bk_sb[:, od:od + 1])

                # ---- v path: vh = vn @ Wv (bias via softmax rowsum trick) ----
                wv_sb = projw.tile([128, 8, OD], F32R, tag="w")
                nc.sync.dma_start(wv_sb, wv.rearrange("(ko p) o -> p ko o", p=128))
                for tc4 in range(4):
                    vn = _layer_norm_tiles(nc, ln_pools, xv, tc4 * 512, 4,
                                           None, 0)
                    vnT = xtc.tile([128, 8, 512], F32R, tag="xT")
                    _transpose_block(nc, tpsum, vn, vnT)
                    for t in range(4):
                        ps = ppsum.tile([128, 512], F32, tag="proj")
                        for ko in range(8):
                            nc.tensor.matmul(
                                ps, vnT[:, ko, t * 128:(t + 1) * 128],
                                wv_sb[:, ko, :], start=(ko == 0), stop=(ko == 7))
                        nc.any.tensor_copy(vh[:, tc4 * 4 + t, :], ps)

            # ================= attention =================
            with (
                tc.tile_pool(name="attc", bufs=1) as attc,
                tc.tile_pool(name="attw", bufs=3) as attw,
                tc.tile_pool(name="attTp", bufs=2) as attTp,
                tc.tile_pool(name="sstat", bufs=6) as sstat,
                tc.tile_pool(name="spsum", bufs=2, space="PSUM") as spsum,
                tc.tile_pool(name="tpsum2", bufs=2, space="PSUM") as tpsum2,
                tc.tile_pool(name="cpsum", bufs=2, space="PSUM") as cpsum,
            ):
                maskb_sb = attc.tile([1, TK], F32R)
                nc.sync.dma_start(maskb_sb, maskb[:, :])
                for h in range(NH):
                    od, hp = h // 2, h % 2
                    attT = attTp.tile([128, TK // 128, TQ], F32R, tag="attT")
                    for qt in range(4):
                        att_t = attw.tile([128, TK], F32R, tag="att")
                        sums = sstat.tile([128, 4], F32, tag="sums")
                        for tkc in range(4):
                            ps = spsum.tile([128, 512], F32, tag="score")
                            nc.tensor.matmul(
                                ps,
                                qhT[hp * 64:(hp + 1) * 64, od,
                                    qt * 128:(qt + 1) * 128],
                                khT[hp * 64:(hp + 1) * 64, od,
                                    tkc * 512:(tkc + 1) * 512],
                                start=True, stop=False)
                            nc.tensor.matmul(
                                ps, ones_sb,
                                maskb_sb[:, tkc * 512:(tkc + 1) * 512],
                                start=False, stop=True, skip_group_check=True)
                            nc.scalar.activation(
                                att_t[:, tkc * 512:(tkc + 1) * 512], ps,
                                ActF.Exp, accum_out=sums[:, tkc:tkc + 1])
                        rinv = sstat.tile([128, 1], F32, tag="rinv")
                        nc.vector.reduce_sum(rinv, sums, axis=mybir.AxisListType.X)
                        nc.vector.reciprocal(rinv, rinv)
                        nc.vector.tensor_scalar_mul(att_t, att_t, rinv)
                        nc.sync.dma_start(
                            att[h, qt * 128:(qt + 1) * 128, :], att_t)
                        # transpose att tile -> attT[:, :, qt*128:...]
                        for g in range(4):
                            pt = tpsum2.tile([128, 4, 128], F32R, tag="tps2")
                            for j in range(4):
                                nc.tensor.transpose(
                                    pt[:, j],
                                    att_t[:, (g * 4 + j) * 128:
                                          (g * 4 + j + 1) * 128],
                                    ident)
                            nc.any.tensor_copy(
                                attT[:, g * 4:(g + 1) * 4,
                                     qt * 128:(qt + 1) * 128], pt)
                    # PV: ctx^T[h] = vh[:, :, 64h:64h+64]^T @ attT
                    cps = cpsum.tile([64, TQ], F32, tag="ctx")
                    for ko in range(TK // 128):
                        nc.tensor.matmul(cps, vh[:, ko, h * 64:(h + 1) * 64],
                                         attT[:, ko, :],
                                         start=(ko == 0), stop=(ko == TK // 128 - 1))
                    nc.any.tensor_scalar_add(ctxT[:, h, :], cps,
                                             bv_sb[:, h:h + 1])

            # ================= output projection =================
            with (
                tc.tile_pool(name="wo_p", bufs=1) as wo_p,
                tc.tile_pool(name="out_sb", bufs=3) as out_sb,
                tc.tile_pool(name="opsum", bufs=2, space="PSUM") as opsum,
            ):
                wo_sb = wo_p.tile([64, NH, D], F32R)
                nc.sync.dma_start(wo_sb, wo.rearrange("(h d) n -> d h n", d=64))
                for m in range(8):
                    ps = opsum.tile([128, 512], F32, tag="out")
                    for h in range(NH):
                        nc.tensor.matmul(ps, wo_sb[:, h, m * 128:(m + 1) * 128],
                                         ctxT[:, h, :],
                                         start=(h == 0), stop=(h == NH - 1))
                    ot = out_sb.tile([128, 512], F32R, tag="outt")
                    nc.any.tensor_copy(ot, ps)
                    nc.sync.dma_start(outT[m * 128:(m + 1) * 128, :], ot)

    nc.compile()
    return nc


def _get_nc():
    global _CACHED_NC
    if _CACHED_NC is None:
        _CACHED_NC = build_nc()
    return _CACHED_NC


def _host_prep(q, k, v, mask, aux_score, ln_q_g, ln_q_b, ln_k_g, ln_k_b,
               ln_v_g, ln_v_b, Wq, bq, Wk, bk, Wv, bv, Wo, bo):
    """Fold LN affines into weights; build per-core input maps."""
    f32 = np.float32
    Wq_e = (ln_q_g[:, None] * Wq).astype(f32)
    Wk_e = (ln_k_g[:, None] * Wk).astype(f32)
    Wv_e = (ln_v_g[:, None] * Wv).astype(f32)
    bq_e = (bq + ln_q_b @ Wq).astype(f32)
    bk_e = (bk + ln_k_b @ Wk).astype(f32)
    bv_e = (bv + ln_v_b @ Wv).astype(f32)
    ones1 = np.ones((1, 128), f32)
    identin = np.eye(128, dtype=f32)
    in_maps = []
    for c in range(8):
        b, hg = c // 2, c % 2
        sl = slice(hg * 512, (hg + 1) * 512)
        mb = np.where(mask[b, 0] == 0, f32(-30000.0), f32(0.0))[None, :]
        in_maps.append({
            "xq": np.ascontiguousarray(q[b]),
            "xk": np.ascontiguousarray(k[b]),
            "xv": np.ascontiguousarray(v[b]),
            "aux8": np.ascontiguousarray(aux_score[b] / f32(8.0)),
            "maskb": np.ascontiguousarray(mb),
            "ones1": ones1,
            "identin": identin,
            "wq": np.ascontiguousarray(Wq_e[:, sl]),
            "wk": np.ascontiguousarray(Wk_e[:, sl]),
            "wv": np.ascontiguousarray(Wv_e[:, sl]),
            "wo": np.ascontiguousarray(Wo[sl, :].astype(f32)),
            "bq": np.ascontiguousarray(bq_e[sl]),
            "bk": np.ascontiguousarray(bk_e[sl]),
            "bv": np.ascontiguousarray(bv_e[sl]),
        })
    return in_maps


def run_device(in_maps):
    from concourse.bass_utils import run_bass_kernel_spmd
    nc = _get_nc()
    return run_bass_kernel_spmd(nc, in_maps, core_ids=list(range(8))).results


def kernel(**inputs):
    inputs = {k2: np.asarray(v2) for k2, v2 in inputs.items()}
    in_maps = _host_prep(**inputs)
    results = run_device(in_maps)

    bo = inputs["bo"].astype(np.float32)
    out = np.zeros((4, TQ, D), np.float32)
    att_full = np.zeros((4, 16, TQ, TK), np.float32)
    for c in range(8):
        b, hg = c // 2, c % 2
        out[b] += results[c]["outT"].T
        att_full[b, hg * NH:(hg + 1) * NH] = results[c]["att"]
    out += bo[None, None, :]
    return out, att_full


# revision 17
# speedup vs baseline: 93.0121x; 93.0121x over previous
"""Multi-head cross-attention (LN + QKV proj + masked/scaled softmax attention
+ output proj) on 8 Trainium2 NeuronCores.

Sharding: core c handles (batch b = c//2, head-group hg = c%2).  Each core
computes LayerNorm + projections for its batch restricted to its 8 heads
(tensor-parallel split of Wq/Wk/Wv output dim and Wo input dim), full
attention for those heads (att written directly), and a partial output
projection.  Host sums the two partial outputs per batch and adds bo.

Device math (fp32 data, float32r matmuls ~ tf32-class, fp32 accumulate):
  LN affine (g, b) is folded into the projection weights on host:
    LN(x; g, b) @ W + c  ==  LNplain(x) @ (g[:, None] * W) + (b @ W + c)
  aux_score/8 is folded into the per-token LN scale of the k path.
  The key-mask becomes an additive -30000 row via a K=1 matmul so masked
  exp() underflows to exactly 0 (matching reference where(m, 0, att)).
"""
import sys

sys.path.insert(0, "/opt/trn_rl_repo")

import numpy as np

import concourse.bass as bass
import concourse.tile as tile
from concourse import bacc, mybir
F32 = mybir.dt.float32
F32R = mybir.dt.float32r
ActF = mybir.ActivationFunctionType
Alu = mybir.AluOpType

TQ = 512          # q tokens per batch
TK = 2048         # kv tokens per batch
D = 1024          # model dim
HD = 64           # head dim
NH = 8            # heads per core
OD = 512          # projected width per core (8 heads * 64)
EPS = 1e-5

_CACHED_NC = None


def _layer_norm_tiles(nc, pools, x_dram, tok0, ntiles, aux_sb, aux_col0):
    """LN over `ntiles` [128, D] token tiles starting at token tok0.

    Returns list of f32r normalized tiles.  If aux_sb is not None, the
    per-token scale r is multiplied by aux_sb[:, aux_col0 + i] (aux/8 fold).
    """
    xpool, npool, spool, eps_sb = pools
    out = []
    for i in range(ntiles):
        xt = xpool.tile([128, D], F32, tag="ln_x")
        nc.sync.dma_start(xt, x_dram[tok0 + i * 128: tok0 + (i + 1) * 128, :])
        stats = spool.tile([128, 2, 6], F32, tag="ln_stats")
        nc.vector.bn_stats(stats[:, 0], xt[:, 0:512])
        nc.vector.bn_stats(stats[:, 1], xt[:, 512:1024])
        mv = spool.tile([128, 2], F32, tag="ln_mv")
        nc.vector.bn_aggr(mv, stats)
        # r = 1/sqrt(var + eps)  (* aux/8 for the k path)
        r = spool.tile([128, 1], F32, tag="ln_r")
        nc.scalar.activation(r, mv[:, 1:2], ActF.Sqrt, bias=eps_sb)
        nc.vector.reciprocal(r, r)
        if aux_sb is not None:
            nc.vector.tensor_tensor(r, r, aux_sb[:, aux_col0 + i: aux_col0 + i + 1],
                                    Alu.mult)
        # bias = -mean * r
        bls = spool.tile([128, 1], F32, tag="ln_b")
        nc.vector.tensor_scalar(bls, mv[:, 0:1], r, -1.0, Alu.mult, Alu.mult)
        xn = npool.tile([128, D], F32R, tag="ln_xn")
        nc.scalar.activation(xn, xt, ActF.Identity, bias=bls, scale=r)
        out.append(xn)
    return out


def _transpose_block(nc, tpsum, xn_tiles, dstT):
    """Transpose 4 [128, D] f32r tiles into dstT [128, D//128, 512]."""
    ident = nc._ident_f32r
    for ko in range(D // 128):
        pt = tpsum.tile([128, 4, 128], F32R, tag="tps")
        for j, xn in enumerate(xn_tiles):
            nc.tensor.transpose(pt[:, j], xn[:, ko * 128:(ko + 1) * 128], ident)
        nc.any.tensor_copy(dstT[:, ko, :], pt)


def build_nc():
    nc = bacc.Bacc("TRN2", target_bir_lowering=False, num_devices=8)

    xq = nc.dram_tensor("xq", [TQ, D], F32, kind="ExternalInput")
    xk = nc.dram_tensor("xk", [TK, D], F32, kind="ExternalInput")
    xv = nc.dram_tensor("xv", [TK, D], F32, kind="ExternalInput")
    aux8 = nc.dram_tensor("aux8", [TK], F32, kind="ExternalInput")
    maskb = nc.dram_tensor("maskb", [1, TK], F32R, kind="ExternalInput")
    ones1 = nc.dram_tensor("ones1", [1, 128], F32R, kind="ExternalInput")
    identin = nc.dram_tensor("identin", [128, 128], F32R, kind="ExternalInput")
    wq = nc.dram_tensor("wq", [D, OD], F32R, kind="ExternalInput")
    wk = nc.dram_tensor("wk", [D, OD], F32R, kind="ExternalInput")
    wv = nc.dram_tensor("wv", [D, OD], F32R, kind="ExternalInput")
    wo = nc.dram_tensor("wo", [OD, D], F32R, kind="ExternalInput")
    bq = nc.dram_tensor("bq", [OD], F32, kind="ExternalInput")
    bkrow = nc.dram_tensor("bkrow", [1, OD], F32R, kind="ExternalInput")
    aux8r = nc.dram_tensor("aux8r", [1, TK], F32R, kind="ExternalInput")
    bv = nc.dram_tensor("bv", [OD], F32, kind="ExternalInput")

    att = nc.dram_tensor("att", [NH, TQ, TK], F32R, kind="ExternalOutput")
    outT = nc.dram_tensor("outT", [D, TQ], F32R, kind="ExternalOutput")

    with tile.TileContext(nc) as tc:
        with (
            tc.tile_pool(name="consts", bufs=1) as consts,
            tc.tile_pool(name="persist", bufs=1) as persist,
        ):
            # ---- constants ----
            eps_sb = consts.tile([128, 1], F32)
            nc.vector.memset(eps_sb, EPS)
            ident = consts.tile([128, 128], F32R)
            nc.sync.dma_start(ident, identin[:, :])
            nc._ident_f32r = ident
            ones_sb = consts.tile([1, 128], F32R)
            nc.sync.dma_start(ones_sb, ones1[:, :])
            aux_sb = consts.tile([128, TK // 128], F32)
            nc.sync.dma_start(aux_sb, aux8.rearrange("(t p) -> p t", p=128))
            bq_sb = consts.tile([128, 4], F32)
            nc.sync.dma_start(bq_sb, bq.rearrange("(o p) -> p o", p=128))
            bkrow_sb = consts.tile([1, OD], F32R)
            nc.sync.dma_start(bkrow_sb, bkrow[:, :])
            aux8r_sb = consts.tile([1, TK], F32R)
            nc.sync.dma_start(aux8r_sb, aux8r[:, :])
            bv_sb = consts.tile([64, NH], F32)
            nc.sync.dma_start(bv_sb, bv.rearrange("(h d) -> d h", d=64))

            # ---- persistent activations ----
            qhT = persist.tile([128, 4, TQ], F32R)      # [od%128, od//128, q]
            khT = persist.tile([128, 4, TK], F32R)      # [od%128, od//128, tk]
            vh = persist.tile([128, TK // 128, OD], F32R)  # [tk%128, tk//128, od]
            ctxT = persist.tile([64, NH, TQ], F32R)     # [hd, head, q]

            # ================= projections =================
            with (
                tc.tile_pool(name="projw", bufs=2) as projw,
                tc.tile_pool(name="lnx", bufs=3) as lnx,
                tc.tile_pool(name="lnn", bufs=4) as lnn,
                tc.tile_pool(name="lns", bufs=8) as lns,
                tc.tile_pool(name="xt_chunk", bufs=2) as xtc,
                tc.tile_pool(name="tpsum", bufs=2, space="PSUM") as tpsum,
                tc.tile_pool(name="ppsum", bufs=2, space="PSUM") as ppsum,
            ):
                # q/k/v weights rotate through 2 shared slots (tag "w")
                wq_sb = projw.tile([128, 8, OD], F32R, tag="w")
                nc.sync.dma_start(wq_sb, wq.rearrange("(ko p) o -> p ko o", p=128))

                ln_pools = (lnx, lnn, lns, eps_sb)

                # ---- q path: LN -> transpose -> qhT = Wq^T @ qn^T (+bq) ----
                qn = _layer_norm_tiles(nc, ln_pools, xq, 0, 4, None, 0)
                qnT = xtc.tile([128, 8, TQ], F32R, tag="xT")
                _transpose_block(nc, tpsum, qn, qnT)
                for od in range(4):
                    ps = ppsum.tile([128, 512], F32, tag="proj")
                    for ko in range(8):
                        nc.tensor.matmul(ps, wq_sb[:, ko, od * 128:(od + 1) * 128],
                                         qnT[:, ko, :],
                                         start=(ko == 0), stop=(ko == 7))
                    nc.any.tensor_scalar_add(qhT[:, od, :], ps,
                                             bq_sb[:, od:od + 1])

                # ---- k path (aux/8 folded), per 512-token chunk ----
                wk_sb = projw.tile([128, 8, OD], F32R, tag="w")
                nc.sync.dma_start(wk_sb, wk.rearrange("(ko p) o -> p ko o", p=128))
                for tc4 in range(4):
                    kn = _layer_norm_tiles(nc, ln_pools, xk, tc4 * 512, 4,
                                           aux_sb, tc4 * 4)
                    knT = xtc.tile([128, 8, 512], F32R, tag="xT")
                    _transpose_block(nc, tpsum, kn, knT)
                    for od in range(4):
                        ps = ppsum.tile([128, 512], F32, tag="proj")
                        for ko in range(8):
                            nc.tensor.matmul(
                                ps, wk_sb[:, ko, od * 128:(od + 1) * 128],
                                knT[:, ko, :], start=(ko == 0), stop=False)
                        # ref scales bk by aux/8 too: kh^T += bk ⊗ aux/8
                        nc.tensor.matmul(
                            ps, bkrow_sb[:, od * 128:(od + 1) * 128],
                            aux8r_sb[:, tc4 * 512:(tc4 + 1) * 512],
                            start=False, stop=True, skip_group_check=True)
                        nc.any.tensor_copy(
                            khT[:, od, tc4 * 512:(tc4 + 1) * 512], ps)

                # ---- v path: vh = vn @ Wv (bias via softmax rowsum trick) ----
                wv_sb = projw.tile([128, 8, OD], F32R, tag="w")
                nc.sync.dma_start(wv_sb, wv.rearrange("(ko p) o -> p ko o", p=128))
                for tc4 in range(4):
                    vn = _layer_norm_tiles(nc, ln_pools, xv, tc4 * 512, 4,
                                           None, 0)
                    vnT = xtc.tile([128, 8, 512], F32R, tag="xT")
                    _transpose_block(nc, tpsum, vn, vnT)
                    for t in range(4):
                        ps = ppsum.tile([128, 512], F32, tag="proj")
                        for ko in range(8):
                            nc.tensor.matmul(
                                ps, vnT[:, ko, t * 128:(t + 1) * 128],
                                wv_sb[:, ko, :], start=(ko == 0), stop=(ko == 7))
                        nc.any.tensor_copy(vh[:, tc4 * 4 + t, :], ps)

            # ================= attention =================
            with (
                tc.tile_pool(name="attc", bufs=1) as attc,
                tc.tile_pool(name="attw", bufs=3) as attw,
                tc.tile_pool(name="attTp", bufs=2) as attTp,
                tc.tile_pool(name="sstat", bufs=6) as sstat,
                tc.tile_pool(name="spsum", bufs=2, space="PSUM") as spsum,
                tc.tile_pool(name="tpsum2", bufs=2, space="PSUM") as tpsum2,
                tc.tile_pool(name="cpsum", bufs=2, space="PSUM") as cpsum,
            ):
                maskb_sb = attc.tile([1, TK], F32R)
                nc.sync.dma_start(maskb_sb, maskb[:, :])
                for h in range(NH):
                    od, hp = h // 2, h % 2
                    attT = attTp.tile([128, TK // 128, TQ], F32R, tag="attT")
                    for qt in range(4):
                        att_t = attw.tile([128, TK], F32R, tag="att")
                        sums = sstat.tile([128, 4], F32, tag="sums")
                        for tkc in range(4):
                            ps = spsum.tile([128, 512], F32, tag="score")
                            nc.tensor.matmul(
                                ps,
                                qhT[hp * 64:(hp + 1) * 64, od,
                                    qt * 128:(qt + 1) * 128],
                                khT[hp * 64:(hp + 1) * 64, od,
                                    tkc * 512:(tkc + 1) * 512],
                                start=True, stop=False)
                            nc.tensor.matmul(
                                ps, ones_sb,
                                maskb_sb[:, tkc * 512:(tkc + 1) * 512],
                                start=False, stop=True, skip_group_check=True)
                            nc.scalar.activation(
                                att_t[:, tkc * 512:(tkc + 1) * 512], ps,
                                ActF.Exp, accum_out=sums[:, tkc:tkc + 1])
                        rinv = sstat.tile([128, 1], F32, tag="rinv")
                        nc.vector.reduce_sum(rinv, sums, axis=mybir.AxisListType.X)
                        nc.vector.reciprocal(rinv, rinv)
                        nc.vector.tensor_scalar_mul(att_t, att_t, rinv)
                        nc.sync.dma_start(
                            att[h, qt * 128:(qt + 1) * 128, :], att_t)
                        # transpose att tile -> attT[:, :, qt*128:...]
                        for g in range(4):
                            pt = tpsum2.tile([128, 4, 128], F32R, tag="tps2")
                            for j in range(4):
                                nc.tensor.transpose(
                                    pt[:, j],
                                    att_t[:, (g * 4 + j) * 128:
                                          (g * 4 + j + 1) * 128],
                                    ident)
                            nc.any.tensor_copy(
                                attT[:, g * 4:(g + 1) * 4,
                                     qt * 128:(qt + 1) * 128], pt)
                    # PV: ctx^T[h] = vh[:, :, 64h:64h+64]^T @ attT
                    cps = cpsum.tile([64, TQ], F32, tag="ctx")
                    for ko in range(TK // 128):
                        nc.tensor.matmul(cps, vh[:, ko, h * 64:(h + 1) * 64],
                                         attT[:, ko, :],
                                         start=(ko == 0), stop=(ko == TK // 128 - 1))
                    nc.any.tensor_scalar_add(ctxT[:, h, :], cps,
                                             bv_sb[:, h:h + 1])

            # ================= output projection =================
            with (
                tc.tile_pool(name="wo_p", bufs=1) as wo_p,
                tc.tile_pool(name="out_sb", bufs=3) as out_sb,
                tc.tile_pool(name="opsum", bufs=2, space="PSUM") as opsum,
            ):
                wo_sb = wo_p.tile([64, NH, D], F32R)
                nc.sync.dma_start(wo_sb, wo.rearrange("(h d) n -> d h n", d=64))
                for m in range(8):
                    ps = opsum.tile([128, 512], F32, tag="out")
                    for h in range(NH):
                        nc.tensor.matmul(ps, wo_sb[:, h, m * 128:(m + 1) * 128],
                                         ctxT[:, h, :],
                                         start=(h == 0), stop=(h == NH - 1))
                    ot = out_sb.tile([128, 512], F32R, tag="outt")
                    nc.any.tensor_copy(ot, ps)
                    nc.sync.dma_start(outT[m * 128:(m + 1) * 128, :], ot)

    nc.compile()
    return nc


def _get_nc():
    global _CACHED_NC
    if _CACHED_NC is None:
        _CACHED_NC = build_nc()
    return _CACHED_NC


def _host_prep(q, k, v, mask, aux_score, ln_q_g, ln_q_b, ln_k_g, ln_k_b,
               ln_v_g, ln_v_b, Wq, bq, Wk, bk, Wv, bv, Wo, bo):
    """Fold LN affines into weights; build per-core input maps."""
    f32 = np.float32
    Wq_e = (ln_q_g[:, None] * Wq).astype(f32)
    Wk_e = (ln_k_g[:, None] * Wk).astype(f32)
    Wv_e = (ln_v_g[:, None] * Wv).astype(f32)
    bq_e = (bq + ln_q_b @ Wq).astype(f32)
    bk_e = (bk + ln_k_b @ Wk).astype(f32)
    bv_e = (bv + ln_v_b @ Wv).astype(f32)
    ones1 = np.ones((1, 128), f32)
    identin = np.eye(128, dtype=f32)
    in_maps = []
    for c in range(8):
        b, hg = c // 2, c % 2
        sl = slice(hg * 512, (hg + 1) * 512)
        mb = np.where(mask[b, 0] == 0, f32(-30000.0), f32(0.0))[None, :]
        in_maps.append({
            "xq": np.ascontiguousarray(q[b]),
            "xk": np.ascontiguousarray(k[b]),
            "xv": np.ascontiguousarray(v[b]),
            "aux8": np.ascontiguousarray(aux_score[b] / f32(8.0)),
            "maskb": np.ascontiguousarray(mb),
            "ones1": ones1,
            "identin": identin,
            "wq": np.ascontiguousarray(Wq_e[:, sl]),
            "wk": np.ascontiguousarray(Wk_e[:, sl]),
            "wv": np.ascontiguousarray(Wv_e[:, sl]),
            "wo": np.ascontiguousarray(Wo[sl, :].astype(f32)),
            "bq": np.ascontiguousarray(bq_e[sl]),
            "bkrow": np.ascontiguousarray(bk_e[sl][None, :]),
            "aux8r": np.ascontiguousarray((aux_score[b] / f32(8.0))[None, :]),
            "bv": np.ascontiguousarray(bv_e[sl]),
        })
    return in_maps


_RUNNER = None


def _get_runner():
    """Build (once) a cached jitted 8-core SPMD executor for the NEFF."""
    global _RUNNER
    if _RUNNER is not None:
        return _RUNNER
    import jax
    from jax.sharding import Mesh, PartitionSpec
    from jax.experimental.shard_map import shard_map
    from concourse import mybir as _mybir
    from concourse.bass2jax import (_bass_exec_p, install_neuronx_cc_hook,
                                    partition_id_tensor)

    nc = _get_nc()
    install_neuronx_cc_hook()
    partition_name = nc.partition_id_tensor.name if nc.partition_id_tensor else None

    in_names, out_names, out_avals, zero_outs = [], [], [], []
    for alloc in nc.m.functions[0].allocations:
        if not isinstance(alloc, _mybir.MemoryLocationSet):
            continue
        name = alloc.memorylocations[0].name
        if alloc.kind == "ExternalInput":
            if name != partition_name:
                in_names.append(name)
        elif alloc.kind == "ExternalOutput":
            shape = tuple(alloc.tensor_shape)
            dtype = _mybir.dt.np(alloc.dtype)
            out_names.append(name)
            out_avals.append(jax.core.ShapedArray(shape, dtype))
            zero_outs.append(np.zeros(shape, dtype))
    n_params = len(in_names)
    all_names = list(in_names) + list(out_names)
    if partition_name is not None:
        all_names.append(partition_name)

    def _body(*args):
        operands = list(args)
        if partition_name is not None:
            operands.append(partition_id_tensor())
        return tuple(_bass_exec_p.bind(
            *operands,
            out_avals=tuple(out_avals),
            in_names=tuple(all_names),
            out_names=tuple(out_names),
            lowering_input_output_aliases=(),
            sim_require_finite=True,
            sim_require_nnan=True,
            nc=nc,
        ))

    devices = jax.devices()[:8]
    mesh = Mesh(np.asarray(devices), ("core",))
    n_out = len(out_names)
    fn = jax.jit(shard_map(
        _body, mesh=mesh,
        in_specs=(PartitionSpec("core"),) * (n_params + n_out),
        out_specs=(PartitionSpec("core"),) * n_out,
        check_rep=False,
    ))
    _RUNNER = (fn, in_names, out_names, out_avals, zero_outs, n_params)
    return _RUNNER


def _concat_inputs(in_maps):
    fn, in_names, out_names, out_avals, zero_outs, n_params = _get_runner()
    concat_in = [np.concatenate([np.asarray(m[name]) for m in in_maps], axis=0)
                 for name in in_names]
    concat_zeros = [np.zeros((8 * z.shape[0], *z.shape[1:]), z.dtype)
                    for z in zero_outs]
    return concat_in + concat_zeros


def run_device(in_maps):
    fn, in_names, out_names, out_avals, zero_outs, n_params = _get_runner()
    out_arrs = fn(*_concat_inputs(in_maps))
    results = []
    host = [np.asarray(a) for a in out_arrs]
    for c in range(8):
        results.append({
            name: host[i].reshape(8, *out_avals[i].shape)[c]
            for i, name in enumerate(out_names)
        })
    return results


def time_device(in_maps, iters=10):
    """Device-resident timing: inputs staged once, repeated execution."""
    import time
    import jax
    fn, *_ = _get_runner()
    args = [jax.device_put(a) for a in _concat_inputs(in_maps)]
    out = fn(*args)
    jax.block_until_ready(out)
    times = []
    for _ in range(iters):
        t0 = time.perf_counter()
        out = fn(*args)
        jax.block_until_ready(out)
        times.append(time.perf_counter() - t0)
    return times


def kernel(**inputs):
    inputs = {k2: np.asarray(v2) for k2, v2 in inputs.items()}
    in_maps = _host_prep(**inputs)
    results = run_device(in_maps)

    bo = inputs["bo"].astype(np.float32)
    out = np.zeros((4, TQ, D), np.float32)
    att_full = np.zeros((4, 16, TQ, TK), np.float32)
    for c in range(8):
        b, hg = c // 2, c % 2
        out[b] += results[c]["outT"].T
        att_full[b, hg * NH:(hg + 1) * NH] = results[c]["att"]
    out += bo[None, None, :]
    return out, att_full
